# revision 33
# baseline (speedup 1.0000x reference)
"""Adaptive multi-head attention on 8 TRN2 NeuronCores.

Reference computation (B=2, S=2048, E=1024, H=16, D=64):
    gate = sigmoid(head_gates * head_importance) > head_threshold   (per head)
    qkv  = x @ qkv_w.T + qkv_b, masked per head
    attn = softmax(q k^T) v  (+ mean of v over seq), inactive heads contribute 0
    out  = attn_heads @ out_w.T + out_b

Sharding: core c = 4*b + g handles batch b and a group of up to 4 heads.
Head gating is resolved on the host (16 scalar sigmoids); inactive heads
contribute exactly zero to the output, so only active heads are computed.
Each core:
  1. computes qT/kT (feature-major) and v (seq-major) for its heads,
  2. runs flash-style attention  (scores^T tiles -> exp on ACT -> @v with a
     ones-column appended to v so the softmax denominators fall out of the
     same matmul),
  3. AllGathers normalized per-group attention outputs (plus v-means) within
     its batch group of 4 cores,
  4. computes a 256-wide slice of the output projection (v-mean residual is
     folded into the projection bias).
Host reassembles the full [B,S,E] output from the 8 projection slices.

Matmul operands are bf16 (projections, gathered features) or float32r
(attention scores/weights, produced rounded as the BIR verifier requires);
both run at full PE rate. The collective-gated projection passes carry a
tile_wait_until scheduling hint so their DMA waits never head-of-line
block the attention stream.
"""

import ml_dtypes
import numpy as np

import concourse.bass as bass
import concourse.tile as tile
from concourse import bacc
from concourse import mybir
from concourse.bass_utils import run_bass_kernel_spmd

B, S, E, H, D = 2, 2048, 1024, 16, 64
_bf16 = np.dtype(ml_dtypes.bfloat16)
NCORES = 8
F32 = mybir.dt.float32
F32R = mybir.dt.float32r
BF16 = mybir.dt.bfloat16
EC = E // 128   # contraction chunks for the projections
SB = S // 512   # 512-wide seq blocks
STn = S // 128  # 128-wide seq tiles
ESL = E // 4    # 256: output-projection column slice per core

# Set by callers that want profiling; read by test harnesses.
TRACE = False
LAST_RESULT = None

_GRAPH_CACHE = {}


def _build_graph(n_h: int, with_qkv_bias: bool = False) -> bacc.Bacc:
    """Per-core graph. n_h = head slots per group (2 or 4), F = 64*n_h."""
    F = 64 * n_h
    F2 = 2 * F          # q+k feature rows
    FT = 4 * F          # gathered feature rows across the 4 group cores
    AF = mybir.ActivationFunctionType

    nc = bacc.Bacc(num_devices=NCORES)
    xT = nc.declare_dram_parameter("xT", [E, S], BF16, isOutput=False)
    wqkT = nc.declare_dram_parameter("wqkT", [E, F2], BF16, isOutput=False)
    wvT = nc.declare_dram_parameter("wvT", [E, F], BF16, isOutput=False)
    bqk = nc.declare_dram_parameter("bqk", [F2], F32, isOutput=False)
    bv = nc.declare_dram_parameter("bv", [F], F32, isOutput=False)
    woT = nc.declare_dram_parameter("woT", [FT, ESL], BF16, isOutput=False)
    bo = nc.declare_dram_parameter("bo", [ESL], F32, isOutput=False)
    out = nc.declare_dram_parameter("out", [ESL, S], F32, isOutput=True)

    with tile.TileContext(nc) as tc:
        with (
            tc.tile_pool(name="pers", bufs=1) as pers,
            tc.tile_pool(name="dram", bufs=1, space="DRAM") as dram,
            tc.tile_pool(name="dramr", bufs=3, space="DRAM") as dramr,
            tc.tile_pool(name="expp", bufs=10) as expp,
            tc.tile_pool(name="recp", bufs=2) as recp,
            tc.tile_pool(name="repp", bufs=2) as repp,
            tc.tile_pool(name="agp", bufs=4) as agp,
            tc.tile_pool(name="outp", bufs=3) as outp,
            tc.tile_pool(name="ps_mm", bufs=3, space="PSUM") as ps_mm,
            tc.tile_pool(name="ps_acc", bufs=1, space="PSUM") as ps_acc,
            tc.tile_pool(name="ps_sc", bufs=2, space="PSUM") as ps_sc,
            tc.tile_pool(name="ps_o", bufs=2, space="PSUM") as ps_o,
        ):
            # ---- persistent SBUF tensors ----
            xt = [pers.tile([128, S], BF16, name=f"xt{i}", tag=f"xt{i}") for i in range(EC)]
            wqk = [pers.tile([128, F2], BF16, name=f"wqk{i}", tag=f"wqk{i}") for i in range(EC)]
            wv = [pers.tile([128, F], BF16, name=f"wv{i}", tag=f"wv{i}") for i in range(EC)]
            qk = [pers.tile([128, S], F32R, name=f"qk{j}", tag=f"qk{j}") for j in range(n_h)]
            vaug = [pers.tile([128, 65 * n_h], F32R, name=f"va{i}", tag=f"va{i}") for i in range(STn)]
            vaug_b = [pers.tile([128, 65 * n_h], BF16, name=f"vb{i}", tag=f"vb{i}") for i in range(STn)]
            att = [pers.tile([128, S], BF16, name=f"att{j}", tag=f"att{j}") for j in range(n_h // 2)]
            wo = [pers.tile([128, ESL], BF16, name=f"wo{i}", tag=f"wo{i}") for i in range(2 * n_h)]
            bqk_raw = pers.tile([1, F2], F32, tag="bqk_raw")
            bqk_row = pers.tile([1, F2], F32R, tag="bqk_row")
            bo_row = pers.tile([1, ESL], F32, tag="bo_row")
            bv_raw = pers.tile([1, F], F32, tag="bv_raw")
            bv_sb = pers.tile([1, F], F32R, tag="bv")
            ones_row = pers.tile([1, 512], F32R, tag="ones_row")
            ones_col = pers.tile([128, 1], F32R, tag="ones_col")
            ones_rraw = pers.tile([1, 512], F32, tag="ones_rraw")
            ones_craw = pers.tile([128, 1], F32, tag="ones_craw")
            ones4 = pers.tile([128, n_h, 1], F32, tag="ones4")
            bias_row = pers.tile([1, ESL], F32R, tag="bias_row")
            vs_row = pers.tile([1, 65 * n_h], F32, tag="vs_row")
            vmg = [pers.tile([128, 1], BF16, name=f"vmg{i}", tag=f"vmg{i}") for i in range(2 * n_h)]

            # DRAM bounce buffers for the collective: per-core payload is
            # [F, S+1] = attention output rows plus a v-mean column.
            bin_h = [dram.tile([64, S + 1], BF16, name=f"bin{c}",
                               tag=f"bin{c}") for c in range(n_h)]
            bout_h = [dram.tile([256, S + 1], BF16, name=f"bout{c}",
                                tag=f"bout{c}") for c in range(n_h)]

            # ---- input DMAs ----
            for i in range(EC):
                nc.sync.dma_start(out=xt[i], in_=xT[i * 128:(i + 1) * 128, :])
                nc.sync.dma_start(out=wqk[i], in_=wqkT[i * 128:(i + 1) * 128, :])
                nc.sync.dma_start(out=wv[i], in_=wvT[i * 128:(i + 1) * 128, :])
            for i in range(2 * n_h):
                nc.sync.dma_start(out=wo[i], in_=woT[i * 128:(i + 1) * 128, :])
            nc.sync.dma_start(out=bqk_raw, in_=bqk.rearrange("(a b) -> a b", a=1))
            nc.vector.tensor_copy(bqk_row, bqk_raw)
            nc.sync.dma_start(out=bo_row, in_=bo.rearrange("(a b) -> a b", a=1))
            nc.sync.dma_start(out=bv_raw, in_=bv.rearrange("(a b) -> a b", a=1))
            nc.vector.tensor_copy(bv_sb, bv_raw)
            nc.vector.memset(ones_rraw, 1.0)
            nc.vector.memset(ones_craw, 1.0)
            nc.vector.memset(ones4, 1.0)
            nc.vector.tensor_copy(ones_row, ones_rraw)
            nc.vector.tensor_copy(ones_col, ones_craw)

            def qk_proj(j):
                # qk[j][:, s] = (W x)^T, feature-major; bias (when nonzero)
                # folded into the accumulation as a rank-1 update
                for sb in range(SB):
                    ps = ps_mm.tile([128, 512], F32, name="qkps", tag="mm")
                    for e in range(EC):
                        nc.tensor.matmul(
                            ps,
                            wqk[e][:, j * 128:(j + 1) * 128],
                            xt[e][:, sb * 512:(sb + 1) * 512],
                            start=(e == 0),
                            stop=(not with_qkv_bias and e == EC - 1),
                        )
                    if with_qkv_bias:
                        nc.tensor.matmul(
                            ps, bqk_row[0:1, j * 128:(j + 1) * 128], ones_row,
                            start=False, stop=True,
                        )
                    nc.vector.tensor_copy(
                        qk[j][:, sb * 512:(sb + 1) * 512], ps
                    )

            def v_proj(interleave_att0=False):
                # v (seq-major) + ones column + running column sums; head 0's
                # first two attention blocks are emitted per-st so exp starts
                # as soon as the first v tile lands
                pos = ([ps_o.tile([65, 512], F32, name="po", tag="po")
                        for _ in range(2)] if interleave_att0 else [])
                vs_ps = ps_acc.tile([1, 65 * n_h], F32, name="vsps", tag="acc")
                for st in range(STn):
                    psv = ps_mm.tile([128, 512], F32, name="vps", tag="mm")
                    for e in range(EC):
                        nc.tensor.matmul(
                            psv[:, 0:F],
                            xt[e][:, st * 128:(st + 1) * 128],
                            wv[e],
                            start=(e == 0),
                            stop=(not with_qkv_bias and e == EC - 1),
                        )
                    if with_qkv_bias:
                        # bias via rank-1 update: ones(s) x bv(f)
                        nc.tensor.matmul(
                            psv[:, 0:F], ones_row[:, 0:128], bv_sb,
                            start=False, stop=True,
                        )
                    va65 = vaug[st].rearrange("p (h x) -> p h x", x=65)
                    nc.vector.tensor_copy(
                        va65[:, :, 0:64], psv[:, 0:F].rearrange("p (h x) -> p h x", x=64)
                    )
                    nc.vector.tensor_copy(va65[:, :, 64:65], ones4)
                    nc.vector.tensor_copy(vaug_b[st], vaug[st])
                    # column sums of v (for the v-mean residual)
                    nc.tensor.matmul(
                        vs_ps, ones_col, vaug[st],
                        start=(st == 0), stop=(st == STn - 1),
                    )
                    if interleave_att0:
                        att_step(0, 0, st, pos[0])
                        att_step(0, 1, st, pos[1])
                if interleave_att0:
                    att_norm(0, 0, pos[0])
                    att_norm(0, 1, pos[1])
                nc.vector.tensor_scalar_mul(vs_row, vs_ps, 1.0 / S)
                vm_c = pers.tile([1, F], BF16, tag="vm_c")
                nc.vector.tensor_copy(
                    vm_c.rearrange("p (h x) -> p h x", x=64),
                    vs_row.rearrange("p (h x) -> p h x", x=65)[:, :, 0:64],
                )
                for c in range(n_h):
                    nc.sync.dma_start(
                        out=bin_h[c][:, S:S + 1],
                        in_=vm_c[0:1, 64 * c:64 * c + 64],
                    )

            def att_step(h, sq, st, po):
                qt = qk[h // 2][(h % 2) * 64:(h % 2) * 64 + 64, :]
                kt = qk[nq + h // 2][(h % 2) * 64:(h % 2) * 64 + 64, :]
                ps = ps_sc.tile([128, 512], F32, name="sc", tag="sc")
                nc.tensor.matmul(
                    ps,
                    kt[:, st * 128:(st + 1) * 128],
                    qt[:, sq * 512:(sq + 1) * 512],
                    start=True, stop=True,
                )
                ex = expp.tile([128, 512], BF16, name="ex", tag="ex")
                nc.scalar.activation(ex, ps, AF.Exp)
                nc.tensor.matmul(
                    po,
                    vaug_b[st][:, h * 65:(h + 1) * 65],
                    ex,
                    start=(st == 0), stop=(st == STn - 1),
                )

            def att_norm(h, sq, po):
                rc = recp.tile([1, 512], F32, name="rc", tag="rc")
                nc.vector.reciprocal(rc, po[64:65, :])
                rcd = dramr.tile([1, 512], F32, name="rcd", tag="rcd")
                nc.sync.dma_start(out=rcd, in_=rc)
                rep = repp.tile([64, 512], F32, name="rep", tag="rep")
                nc.sync.dma_start(out=rep, in_=rcd.to_broadcast([64, 512]))
                nc.vector.tensor_mul(
                    att[h // 2][(h % 2) * 64:(h % 2) * 64 + 64,
                                sq * 512:(sq + 1) * 512],
                    po[0:64, :], rep,
                )

            def attention(h, sqs=None):
                for sq in sqs if sqs is not None else range(SB):
                    po = ps_o.tile([65, 512], F32, name="po", tag="po")
                    for st in range(STn):
                        att_step(h, sq, st, po)
                    att_norm(h, sq, po)

            def gather(h):
                # per-head AllGather into a contiguous per-slot buffer so
                # comm overlaps attention (collective APs must be contiguous)
                nc.sync.dma_start(
                    out=bin_h[h][:, 0:S],
                    in_=att[h // 2][(h % 2) * 64:(h % 2) * 64 + 64, :],
                )
                nc.gpsimd.collective_compute(
                    "AllGather",
                    mybir.AluOpType.bypass,
                    replica_groups=[[0, 1, 2, 3], [4, 5, 6, 7]],
                    ins=[bin_h[h].opt()],
                    outs=[bout_h[h].opt()],
                )

            nq = n_h // 2

            def proj_bias():
                # bias row = out_b slice + (gathered v-means) @ woT, as [1, ESL]
                for fc in range(2 * n_h):
                    r, w = divmod(fc, n_h // 2)
                    for ci in range(2):
                        c = 2 * w + ci
                        nc.sync.dma_start(
                            out=vmg[fc][64 * ci:64 * ci + 64, :],
                            in_=bout_h[c][64 * r:64 * r + 64, S:S + 1],
                        )
                pc = ps_acc.tile([1, ESL], F32, name="pc", tag="acc")
                for fc in range(2 * n_h):
                    nc.tensor.matmul(
                        pc, vmg[fc], wo[fc],
                        start=(fc == 0), stop=(fc == 2 * n_h - 1),
                    )
                nc.vector.tensor_add(bias_row, pc, bo_row)

            def out_proj_pass(stage):
                # stage "A": all slots but the last two -> part (copy)
                # stage "B1": slot n_h-2 -> part (add)   [skipped for n_h=2]
                # stage "B2": last slot + bias -> out
                if stage == "A":
                    slots = list(range(max(1, n_h - 2)))
                elif stage == "B1":
                    slots = [n_h - 2] if n_h > 2 else []
                else:
                    slots = [n_h - 1]
                if not slots:
                    return
                first = stage == "A"
                nr = 64 * len(slots)
                p0 = (64 * slots[0]) % 128
                for sb in range(SB):
                    ags = []
                    for r in range(4):
                        ag = agp.tile([128, 512], BF16, name="ag",
                                      tag=f"ag_{stage}")
                        for ci, c in enumerate(slots):
                            nc.sync.dma_start(
                                out=ag[p0 + 64 * ci:p0 + 64 * ci + 64, :],
                                in_=bout_h[c][r * 64:(r + 1) * 64,
                                              sb * 512:(sb + 1) * 512],
                            )
                        ags.append(ag[p0:p0 + nr, :])
                    for et in range(2):
                        pp = ps_mm.tile([128, 512], F32, name="pp", tag="mm")
                        for r in range(4):
                            wo_t = wo[(r * F + 64 * slots[0]) // 128]
                            nc.tensor.matmul(
                                pp, wo_t[p0:p0 + nr,
                                         et * 128:(et + 1) * 128],
                                ags[r],
                                start=(r == 0),
                                stop=(stage != "B2" and r == 3),
                            )
                        dst = part[et][:, sb * 512:(sb + 1) * 512]
                        if stage == "A":
                            nc.vector.tensor_copy(dst, pp)
                        elif stage == "B1":
                            nc.vector.tensor_add(dst, dst, pp)
                        else:
                            nc.tensor.matmul(
                                pp,
                                bias_row[0:1, et * 128:(et + 1) * 128],
                                ones_row,
                                start=False, stop=True,
                            )
                            ob = outp.tile([128, 512], F32, name="ob", tag="ob")
                            if n_h > 2:
                                nc.vector.tensor_add(ob, pp, dst)
                            else:
                                nc.vector.tensor_copy(ob, pp)
                            nc.sync.dma_start(
                                out=out[et * 128:(et + 1) * 128,
                                        sb * 512:(sb + 1) * 512],
                                in_=ob,
                            )

            part = [pers.tile([128, S], F32, name=f"part{et}", tag=f"part{et}")
                    for et in range(2)]
            ag_c = {}

            # program order chosen so attention h=0 can start as early as
            # possible and comm/projection overlap the attention tail
            qk_proj(0)
            qk_proj(nq)          # k tile for heads 0..1
            v_proj(interleave_att0=True)
            attention(0, sqs=range(2, SB))
            gather(0)
            for j in range(1, nq):
                qk_proj(j)
                qk_proj(nq + j)
            attention(1)
            gather(1)
            for h in range(2, n_h):
                attention(h)
                gather(h)
            # Hint the scheduler to place the collective-gated projection
            # work after the attention region: its ag-load waits otherwise
            # land early in the PE stream and head-of-line block attention.
            with tc.tile_wait_until(0.18):
                out_proj_pass("A")
                out_proj_pass("B1")
                proj_bias()
                out_proj_pass("B2")
    nc.finalize()
    return nc


def _get_graph(n_h: int, with_qkv_bias: bool) -> bass.Bass:
    key = (n_h, with_qkv_bias)
    if key not in _GRAPH_CACHE:
        _GRAPH_CACHE[key] = _build_graph(n_h, with_qkv_bias)
    return _GRAPH_CACHE[key]


def _shard_inputs(x, qkv_w, qkv_b, out_w, out_b, per_group, n_h):
    F = 64 * n_h
    FT = 4 * F
    head_order = [h for sl in per_group for h in sl]

    woT_full = np.zeros((FT, E), dtype=np.float32)
    for s_i, h in enumerate(head_order):
        if h is not None:
            woT_full[s_i * 64:(s_i + 1) * 64, :] = out_w[:, h * 64:(h + 1) * 64].T

    def _slice_rows(sl, base):
        m = np.zeros((F, E), dtype=np.float32)
        bb = np.zeros((F,), dtype=np.float32)
        for i, h in enumerate(sl):
            if h is not None:
                m[i * 64:(i + 1) * 64] = qkv_w[base + h * 64: base + (h + 1) * 64]
                bb[i * 64:(i + 1) * 64] = qkv_b[base + h * 64: base + (h + 1) * 64]
        return m, bb

    xT = [np.ascontiguousarray(x[b].T) for b in range(B)]
    in_maps = []
    for c in range(NCORES):
        b, g = divmod(c, 4)
        wq, bq = _slice_rows(per_group[g], 0)
        wk, bk = _slice_rows(per_group[g], E)
        wv_, bv_ = _slice_rows(per_group[g], 2 * E)
        in_maps.append({
            "xT": xT[b].astype(_bf16),
            "wqkT": np.ascontiguousarray(np.concatenate([wq, wk], 0).T).astype(_bf16),
            "wvT": np.ascontiguousarray(wv_.T).astype(_bf16),
            "bqk": np.concatenate([bq, bk]),
            "bv": bv_,
            "woT": np.ascontiguousarray(woT_full[:, g * ESL:(g + 1) * ESL]).astype(_bf16),
            "bo": np.ascontiguousarray(out_b[g * ESL:(g + 1) * ESL]),
        })
    return in_maps


def plan(head_gates, head_importance, head_threshold):
    head_gates = np.asarray(head_gates, dtype=np.float32)
    head_importance = np.asarray(head_importance, dtype=np.float32)
    thr = np.float32(np.asarray(head_threshold, dtype=np.float32))
    gate = (1.0 / (1.0 + np.exp(-(head_gates * head_importance),
                                dtype=np.float32))).astype(np.float32)
    active = [int(h) for h in range(H) if gate[h] > thr]
    if not active:
        return None, None
    n_h = 2 if len(active) <= 8 else 4
    slots = [[] for _ in range(4)]
    for i, h in enumerate(active):
        slots[i % 4].append(h)
    per_group = [sl + [None] * (n_h - len(sl)) for sl in slots]
    return n_h, per_group


def kernel(x, qkv_w, qkv_b, out_w, out_b, head_gates, head_importance,
           head_threshold):
    global LAST_RESULT
    x = np.ascontiguousarray(np.asarray(x, dtype=np.float32))
    qkv_w = np.asarray(qkv_w, dtype=np.float32)
    qkv_b = np.asarray(qkv_b, dtype=np.float32)
    out_w = np.asarray(out_w, dtype=np.float32)
    out_b = np.asarray(out_b, dtype=np.float32)

    n_h, per_group = plan(head_gates, head_importance, head_threshold)
    if n_h is None:
        # All heads gated off: attention contributes exactly 0, out = out_b.
        return np.broadcast_to(
            out_b[None, None, :], (B, S, E)).astype(np.float32).copy()

    in_maps = _shard_inputs(x, qkv_w, qkv_b, out_w, out_b, per_group, n_h)
    nc = _get_graph(n_h, bool(np.any(qkv_b)))
    res = run_bass_kernel_spmd(nc, in_maps, list(range(NCORES)), trace=TRACE)
    LAST_RESULT = res

    full = np.empty((B, S, E), dtype=np.float32)
    for b in range(B):
        outT = np.concatenate(
            [res.results[4 * b + g]["out"] for g in range(4)], axis=0)
        full[b] = outT.T
    return full
